# revision 18
# baseline (speedup 1.0000x reference)
"""Trainium2 Bass kernel for chunked causal linear attention (elu+1 feature map).

Reference computation (B=2, N=2048, D=1024, DHAT=512, H=16, F=32, G=64):
    Q = phi(x @ wq + bq), K = phi(x @ wk + bk), V = x @ wv + bv   (per-head split)
    kv_t = cumsum_t(K_t outer V_t);  Z_t = 1/(Q_t . cumsum_t(K)_t + 1e-6)
    out_t = (Q_t . kv_t) * Z_t;  y = out @ wo + bo
with phi(x) = elu(x) + 1 = min(exp(x), max(x + 1, 1)).

Sharding over 8 cores: core c handles batch b = c//4 and heads 4r..4r+3
(r = c%4).  Each core projects its head slice, runs chunk-parallel linear
attention (chunk C=256: intra-chunk masked QK^T + inter-chunk running state
S[f,g]), and computes a PARTIAL output projection through its 256 rows of
wo.  The host sums the 4 partials per batch (the output-projection
all-reduce realized at unshard time) — on-device collectives on this setup
cost ~25x their table values, far more than the whole compute.

All big matmuls run as float32r (full-rate fp32 on the PE at moving>=256).
"""
import os
import sys
import types

sys.path.insert(0, "/opt/trn_rl_repo")

import numpy as np

# ---- problem constants (hardcoded; kernel.py must be self-contained) ----
B, N, D, DHAT, H = 2, 2048, 1024, 512, 16
F = DHAT // H        # 32
G = D // H           # 64
HPC = H // 8         # heads per... (4 cores per batch -> 4 heads/core)
NCORES = 8
CHUNK = 256          # attention chunk along t
NCHUNK = N // CHUNK  # 8
JLOC = 4 * G         # 256 local attention features per core
TB = 512             # projection t-block


def _install_ntff_hook():
    """Register the axon NTFF profiling hook (stub antenv lacks axon_hooks)."""
    if "antenv.axon_hooks" in sys.modules:
        return
    try:
        from trn_agent_boot.trn_boot import _ntff_profile_via_ctypes
        hook = _ntff_profile_via_ctypes("/opt/axon/libaxon_pjrt.so")
    except Exception:
        hook = None
    m = types.ModuleType("antenv.axon_hooks")
    m.get_axon_ntff_profile_hook = lambda: hook
    m.set_axon_ntff_profile_hook = lambda h: None
    sys.modules["antenv.axon_hooks"] = m


def build_nc():
    import concourse.bass as bass
    import concourse.mybir as mybir
    import concourse.tile as tile
    from concourse import bacc

    F32 = mybir.dt.float32
    F32R = mybir.dt.float32r
    AF = mybir.ActivationFunctionType
    ALU = mybir.AluOpType

    nc = bacc.Bacc("TRN2", target_bir_lowering=False, debug=False,
                   num_devices=NCORES)

    # ---- per-core DRAM parameters ----
    xT_e = nc.declare_dram_parameter("xT", [D, N], F32R, isOutput=False)
    wq_e = nc.declare_dram_parameter("wq", [D, 4 * F], F32R, isOutput=False)
    wk_e = nc.declare_dram_parameter("wk", [D, 4 * F], F32R, isOutput=False)
    wv_e = nc.declare_dram_parameter("wv", [D, JLOC], F32R, isOutput=False)
    wo_e = nc.declare_dram_parameter("wo", [JLOC, D], F32R, isOutput=False)
    bq_e = nc.declare_dram_parameter("bq", [4 * F, 1], F32, isOutput=False)
    bk_e = nc.declare_dram_parameter("bk", [4 * F, 1], F32, isOutput=False)
    bv_e = nc.declare_dram_parameter("bv", [JLOC, 1], F32, isOutput=False)
    bo_e = nc.declare_dram_parameter("bo", [1, D], F32, isOutput=False)
    y_e = nc.declare_dram_parameter("y", [N, D], F32, isOutput=True)
    dbg = os.environ.get("KDEBUG")
    if dbg:
        dq_e = nc.declare_dram_parameter("dbg_q", [F, N], F32, isOutput=True)
        dk_e = nc.declare_dram_parameter("dbg_k", [F, N], F32, isOutput=True)
        dv_e = nc.declare_dram_parameter("dbg_v", [G, N], F32, isOutput=True)
        da_e = nc.declare_dram_parameter("dbg_a", [128, N], F32, isOutput=True)

    # host-precomputed causal masks for the two 128-row blocks of a 256 chunk
    m0 = np.zeros((128, CHUNK), np.float32)
    m1 = np.zeros((128, CHUNK), np.float32)
    for s in range(128):
        m0[s, s:] = 1.0
        m1[s, 128 + s:] = 1.0
    mask0_d = nc.inline_tensor(m0, "mask0")
    mask1_d = nc.inline_tensor(m1, "mask1")
    ident_d = nc.inline_tensor(np.eye(128, dtype=np.float32), "identc")
    ones64_d = nc.inline_tensor(np.ones((1, 128), np.float32), "ones64c")
    va0 = np.zeros((128, 128), np.float32)
    va0[:, G] = 1.0
    vainit_d = nc.inline_tensor(va0, "vainitc")
    zeros_d = nc.inline_tensor(np.zeros((128, 128), np.float32), "zerosc")

    with tile.TileContext(nc) as tc:
        with (
            tc.tile_pool(name="persist", bufs=1) as pers,
            tc.tile_pool(name="xin", bufs=8) as xin,
            tc.tile_pool(name="ppool", bufs=2, space="PSUM") as pp,
            tc.tile_pool(name="apool", bufs=3, space="PSUM") as apsum,
            tc.tile_pool(name="spool", bufs=1, space="PSUM") as spsum,
            tc.tile_pool(name="opool", bufs=2, space="PSUM") as opsum,
            tc.tile_pool(name="work", bufs=4) as work,
            tc.tile_pool(name="ssb", bufs=2) as ssb,
            tc.tile_pool(name="ysb", bufs=3) as ysb,
        ):
            # ---- persistent SBUF state ----
            ident = pers.tile([128, 128], F32R)
            nc.sync.dma_start(ident[:], ident_d[:].bitcast(F32R))
            ones64 = pers.tile([1, 128], F32R)
            nc.sync.dma_start(ones64[:], ones64_d[:].bitcast(F32R))


            mask_sb = [pers.tile([128, CHUNK], F32R, tag=f"mask{i}", name=f"mask{i}")
                       for i in range(2)]
            nc.sync.dma_start(mask_sb[0][:], mask0_d[:].bitcast(F32R))
            nc.sync.dma_start(mask_sb[1][:], mask1_d[:].bitcast(F32R))

            KD = D // 128  # 8 contraction tiles
            wq_sb = [pers.tile([128, 4 * F], F32R, tag=f"wq{k}", name=f"wq{k}") for k in range(KD)]
            wk_sb = [pers.tile([128, 4 * F], F32R, tag=f"wk{k}", name=f"wk{k}") for k in range(KD)]
            for k in range(KD):
                nc.sync.dma_start(wq_sb[k][:], wq_e[128 * k:128 * (k + 1), :])
                nc.sync.dma_start(wk_sb[k][:], wk_e[128 * k:128 * (k + 1), :])
            wv_sb = [[pers.tile([128, 128], F32R, tag=f"wv{k}_{mth}", name=f"wv{k}_{mth}")
                      for mth in range(2)] for k in range(KD)]
            for k in range(KD):
                for mth in range(2):
                    nc.sync.dma_start(
                        wv_sb[k][mth][:],
                        wv_e[128 * k:128 * (k + 1), 128 * mth:128 * (mth + 1)])
            wo_sb = [pers.tile([128, D], F32R, tag=f"wo{j}", name=f"wo{j}") for j in range(2)]
            for j in range(2):
                nc.sync.dma_start(wo_sb[j][:], wo_e[128 * j:128 * (j + 1), :])

            bq_sb = pers.tile([4 * F, 1], F32)
            bk_sb = pers.tile([4 * F, 1], F32)
            bv_sb = [pers.tile([128, 1], F32, tag=f"bv{i}", name=f"bv{i}")
                     for i in range(2)]
            nc.sync.dma_start(bq_sb[:], bq_e[:])
            nc.sync.dma_start(bk_sb[:], bk_e[:])
            # bias+1 variants for the relu(u)+1 branch of phi
            bq1_sb = pers.tile([4 * F, 1], F32)
            bk1_sb = pers.tile([4 * F, 1], F32)
            nc.vector.tensor_scalar(bq1_sb[:], bq_sb[:], 1.0, None, op0=ALU.add)
            nc.vector.tensor_scalar(bk1_sb[:], bk_sb[:], 1.0, None, op0=ALU.add)
            for i in range(2):
                nc.sync.dma_start(bv_sb[i][:], bv_e[128 * i:128 * (i + 1), :])
            # bo broadcast across the 128 partitions (partition-step-0 DMA)
            bo_sb = pers.tile([128, D], F32)
            bo_ap = bo_e.ap()
            bo_bcast = bass.AP(tensor=bo_ap.tensor, offset=0,
                               ap=[[0, 128], [1, D]])
            nc.sync.dma_start(bo_sb[:], bo_bcast)

            # per-head feature-major tiles (base partition 0 for PE operands)
            qh = [pers.tile([F, N], F32R, tag=f"qh{h}", name=f"qh{h}")
                  for h in range(4)]
            kh = [pers.tile([F, N], F32R, tag=f"kh{h}", name=f"kh{h}")
                  for h in range(4)]
            vh = [pers.tile([G, N], F32R, tag=f"vh{h}", name=f"vh{h}")
                  for h in range(4)]
            aT = [pers.tile([128, N], F32R, tag=f"aT{i}", name=f"aT{i}") for i in range(2)]
            # t-major V_aug / K tiles, padded to [128,128] for fp32r matmuls;
            # pads initialized once (ones column for V_aug baked into init)
            va_p = [[pers.tile([128, 128], F32R, tag=f"va{h}_{s}",
                               name=f"va{h}_{s}") for s in range(2)]
                    for h in range(4)]
            ktm_p = [[pers.tile([128, 128], F32R, tag=f"ktm{h}_{s}",
                                name=f"ktm{h}_{s}") for s in range(2)]
                     for h in range(4)]
            for h in range(4):
                for s in range(2):
                    nc.sync.dma_start(va_p[h][s][:], vainit_d[:].bitcast(F32R))
                    nc.sync.dma_start(ktm_p[h][s][:], zeros_d[:].bitcast(F32R))

            # ---- projections, feature-major, per t-block ----
            for tb in range(N // TB):
                tsl = slice(TB * tb, TB * (tb + 1))
                xt = [xin.tile([128, TB], F32R, tag="xt", name="xt") for _ in range(KD)]
                for k in range(KD):
                    nc.sync.dma_start(xt[k][:], xT_e[128 * k:128 * (k + 1), tsl])

                for (w_sb, b_sb, b1_sb, dsts) in (
                        (wq_sb, bq_sb, bq1_sb, qh), (wk_sb, bk_sb, bk1_sb, kh)):
                    ps = pp.tile([128, TB], F32, tag="proj")
                    for k in range(KD):
                        nc.tensor.matmul(ps[:], w_sb[k][:], xt[k][:],
                                         start=(k == 0), stop=(k == KD - 1))
                    # phi(u) = min(exp(u), max(u + 1, 1)), bias folded in
                    e_sb = work.tile([128, TB], F32R, tag="phi_e")
                    nc.scalar.activation(e_sb[:], ps[:], AF.Exp, bias=b_sb[:])
                    u_sb = work.tile([128, TB], F32R, tag="phi_u")
                    nc.vector.tensor_scalar(u_sb[:], ps[:], b1_sb[:], 1.0,
                                            op0=ALU.add, op1=ALU.max)
                    for h in range(4):
                        fr = slice(F * h, F * (h + 1))
                        nc.vector.tensor_tensor(dsts[h][:, tsl], e_sb[fr, :],
                                                u_sb[fr, :], op=ALU.min)

                for mth in range(2):
                    ps = pp.tile([128, TB], F32, tag="proj")
                    for k in range(KD):
                        nc.tensor.matmul(ps[:], wv_sb[k][mth][:], xt[k][:],
                                         start=(k == 0), stop=(k == KD - 1))
                    for half in range(2):
                        h = 2 * mth + half
                        gr = slice(G * half, G * (half + 1))
                        nc.scalar.activation(
                            vh[h][:, tsl], ps[gr, :], AF.Identity,
                            bias=bv_sb[mth][gr, :])

            # ---- chunked linear attention, per head ----
            for h in range(4):
                atile = aT[h // 2]
                vrow = 64 * (h % 2)
                s_prev = None
                for i in range(NCHUNK):
                    t0 = CHUNK * i
                    csl = slice(t0, t0 + CHUNK)
                    # --- intra-chunk A^T = (K^T)^T Q^T per 128-row s-block ---
                    am = []
                    for sb2 in range(2):
                        ssl = slice(t0 + 128 * sb2, t0 + 128 * (sb2 + 1))
                        a_ps = apsum.tile([128, CHUNK], F32, tag="A")
                        nc.tensor.matmul(a_ps[:], kh[h][:, ssl], qh[h][:, csl],
                                         start=True, stop=True)
                        am_sb = work.tile([128, CHUNK], F32R, tag="am")
                        nc.vector.tensor_tensor(am_sb[:], a_ps[:],
                                                mask_sb[sb2][:], op=ALU.mult)
                        am.append(am_sb)
                    # --- t-major V_aug (V chunk block + ones column) ---
                    vaug, ktm = [], []
                    for sb2 in range(2):
                        ssl = slice(t0 + 128 * sb2, t0 + 128 * (sb2 + 1))
                        vt_ps = apsum.tile([128, G], F32R, tag="A", name="vt_ps")
                        nc.tensor.transpose(
                            vt_ps[:], vh[h][:, ssl], ident[0:G, 0:G])
                        va = va_p[h][sb2]
                        nc.vector.tensor_copy(va[:, 0:G], vt_ps[:])
                        vaug.append(va)
                        kt_ps = apsum.tile([128, F], F32R, tag="A", name="kt_ps")
                        nc.tensor.transpose(kt_ps[:], kh[h][:, ssl],
                                            ident[0:F, 0:F])
                        km = ktm_p[h][sb2]
                        nc.vector.tensor_copy(km[:, 0:F], kt_ps[:])
                        ktm.append(km)
                    # --- out^T (rows 0..G-1) + denominator (row G) ---
                    o_ps = opsum.tile([128, CHUNK], F32, tag="o", name="o_ps")
                    nc.tensor.matmul(o_ps[:], vaug[0][:], am[0][:],
                                     start=True, stop=False)
                    nc.tensor.matmul(o_ps[:], vaug[1][:], am[1][:],
                                     start=False, stop=(s_prev is None))
                    if s_prev is not None:
                        nc.tensor.matmul(o_ps[:], s_prev[:], qh[h][:, csl],
                                         start=False, stop=True)
                    # --- state update S += K_chunk^T V_aug (SBUF-accumulated) ---
                    if i < NCHUNK - 1:
                        s_ps = spsum.tile([128, 128], F32, tag="S", name="s_ps")
                        nc.tensor.matmul(s_ps[:], ktm[0][:], vaug[0][:],
                                         start=True, stop=False)
                        nc.tensor.matmul(s_ps[:], ktm[1][:], vaug[1][:],
                                         start=False, stop=True)
                        s_sb = ssb.tile([F, 128], F32R, tag="ssb")
                        if s_prev is None:
                            nc.vector.tensor_copy(s_sb[:], s_ps[0:F, :])
                        else:
                            nc.vector.tensor_tensor(s_sb[:], s_ps[0:F, :],
                                                    s_prev[:], op=ALU.add)
                        s_prev = s_sb
                    # --- normalize: attn^T = out^T / (denom + 1e-6) ---
                    den_sb = work.tile([1, CHUNK], F32R, tag="den")
                    nc.vector.tensor_scalar(den_sb[:], o_ps[G:G + 1, :],
                                            1e-6, None, op0=ALU.add)
                    bc_ps = opsum.tile([128, CHUNK], F32, tag="o", name="bc_ps")
                    nc.tensor.matmul(bc_ps[:], ones64[:], den_sb[:],
                                     start=True, stop=True)
                    rec_sb = work.tile([G, CHUNK], F32, tag="rec")
                    nc.vector.reciprocal(rec_sb[:], bc_ps[0:G, :])
                    nc.vector.tensor_tensor(
                        atile[vrow:vrow + G, csl], o_ps[0:G, :], rec_sb[:],
                        op=ALU.mult)

            if dbg:
                nc.sync.dma_start(dq_e[:], qh[0][:].bitcast(F32))
                nc.sync.dma_start(dk_e[:], kh[0][:].bitcast(F32))
                nc.sync.dma_start(dv_e[:], vh[0][:].bitcast(F32))
                nc.sync.dma_start(da_e[:], aT[0][:].bitcast(F32))

            # ---- partial output projection: y[t, e] = attn^T.T @ wo ----
            for tt in range(N // 128):
                tsl = slice(128 * tt, 128 * (tt + 1))
                for eb in range(2):
                    esl = slice(512 * eb, 512 * (eb + 1))
                    y_ps = opsum.tile([128, 512], F32, tag="o", name="y_ps")
                    for j in range(2):
                        nc.tensor.matmul(y_ps[:], aT[j][:, tsl],
                                         wo_sb[j][:, esl],
                                         start=(j == 0), stop=(j == 1))
                    y_sb = ysb.tile([128, 512], F32, tag="ysb")
                    nc.vector.tensor_tensor(y_sb[:], y_ps[:], bo_sb[:, esl],
                                            op=ALU.add)
                    nc.sync.dma_start(y_e[tsl, esl], y_sb[:])

    nc.compile()
    return nc


def make_in_maps(x, wq, bq, wk, bk, wv, bv, wo, bo):
    x = np.asarray(x, np.float32)
    in_maps = []
    for c in range(NCORES):
        b, r = divmod(c, 4)
        in_maps.append({
            "xT": np.ascontiguousarray(x[b].T),
            "wq": np.ascontiguousarray(wq[:, 128 * r:128 * (r + 1)]),
            "wk": np.ascontiguousarray(wk[:, 128 * r:128 * (r + 1)]),
            "wv": np.ascontiguousarray(wv[:, 256 * r:256 * (r + 1)]),
            "wo": np.ascontiguousarray(wo[256 * r:256 * (r + 1), :]),
            "bq": np.ascontiguousarray(bq[128 * r:128 * (r + 1)]).reshape(-1, 1),
            "bk": np.ascontiguousarray(bk[128 * r:128 * (r + 1)]).reshape(-1, 1),
            "bv": np.ascontiguousarray(bv[256 * r:256 * (r + 1)]).reshape(-1, 1),
            "bo": (np.asarray(bo, np.float32) if r == 0
                   else np.zeros_like(bo)).reshape(1, -1),
        })
    return [{k: np.ascontiguousarray(v, np.float32) for k, v in m.items()}
            for m in in_maps]


def assemble(results):
    y = np.zeros((B, N, D), np.float32)
    for c in range(NCORES):
        y[c // 4] += results[c]["y"]
    return y


_NC_CACHE = {}


def run(inputs, trace=False):
    _install_ntff_hook()
    from concourse.bass_utils import run_bass_kernel_spmd
    if "nc" not in _NC_CACHE:
        _NC_CACHE["nc"] = build_nc()
    nc = _NC_CACHE["nc"]
    in_maps = make_in_maps(**inputs)
    res = run_bass_kernel_spmd(nc, in_maps, core_ids=list(range(NCORES)),
                               trace=trace)
    return assemble(res.results), res.exec_time_ns


def kernel(**inputs) -> np.ndarray:
    y, _ = run(inputs, trace=False)
    return y
